# revision 2
# baseline (speedup 1.0000x reference)
"""Trainium2 Bass kernel for nn_NeuronFDAttentionBase (GQA causal attention block).

Self-contained: builds an SPMD Bass/Tile kernel for 8 NeuronCores.
Sharding: core c owns query rows {c, c+8, ...} (FD-style strided) — identical
causal structure on every core; K/V projection replicated; attention computed
in transposed [k, q] orientation so probs feed the PV matmul without
transposes; output projection consumes attn^T directly.
"""

import numpy as np

N_CORES = 8
S = 4096          # sequence length
HID = 4096        # hidden dim
NH = 4            # query heads
D = 128           # head dim
SL = S // N_CORES # 512 local q rows per core
NHT = HID // 128  # 32 h-tiles
NSC = S // 512    # 8 seq chunks
NKT = S // 128    # 32 k-tiles
SCALE = 1.0 / np.sqrt(np.float32(D))
NEG = -1.0e9

_CACHE = {}
LAST_PROFILE = None


def _build(use_f32r=True):
    import concourse.mybir as mybir
    import concourse.tile as tile
    from concourse import bacc
    from concourse.masks import make_identity
    from contextlib import ExitStack

    dt = mybir.dt
    f32 = dt.float32
    fmm = dt.float32r if use_f32r else dt.float32  # matmul operand dtype

    nc = bacc.Bacc("TRN2", target_bir_lowering=False, debug=False,
                   num_devices=N_CORES)

    # ---- DRAM I/O ----
    hsT_d = nc.dram_tensor("hsT", [HID, S], fmm, kind="ExternalInput").ap()
    hsQ_d = nc.dram_tensor("hsQ", [HID, SL], fmm, kind="ExternalInput").ap()
    wq_d = nc.dram_tensor("wq", [HID, NH * D], fmm, kind="ExternalInput").ap()
    wk_d = nc.dram_tensor("wk", [HID, D], fmm, kind="ExternalInput").ap()
    wv_d = nc.dram_tensor("wv", [HID, D], fmm, kind="ExternalInput").ap()
    wo_d = nc.dram_tensor("wo", [NH * D, HID], fmm, kind="ExternalInput").ap()
    mband_d = nc.dram_tensor("mband", [128, 16], f32, kind="ExternalInput").ap()

    out_d = nc.dram_tensor("out", [SL, HID], f32, kind="ExternalOutput").ap()
    ktf_d = nc.dram_tensor("ktf", [D, S], fmm, kind="ExternalOutput").ap()
    vtf_d = nc.dram_tensor("vtf", [D, S], fmm, kind="ExternalOutput").ap()

    hsT_r = hsT_d.rearrange("(i p) s -> i p s", p=128)    # [32, 128, 4096]
    hsQ_r = hsQ_d.rearrange("(i p) q -> i p q", p=128)    # [32, 128, 512]
    wq_r = wq_d.rearrange("(i p) (h d) -> p i h d", p=128, h=NH)
    wk_r = wk_d.rearrange("(i p) d -> p i d", p=128)
    wv_r = wv_d.rearrange("(i p) d -> p i d", p=128)
    wo_r = wo_d.rearrange("(h p) (n c) -> p h n c", p=128, c=512)

    with tile.TileContext(nc) as tc, ExitStack() as ctx:
        # ---- long-lived SBUF pools ----
        kv = ctx.enter_context(tc.tile_pool(name="kv", bufs=1))
        misc = ctx.enter_context(tc.tile_pool(name="misc", bufs=1))
        qtp = ctx.enter_context(tc.tile_pool(name="qtp", bufs=1))
        wbig = ctx.enter_context(tc.tile_pool(name="wbig", bufs=1))
        wkvp = ctx.enter_context(tc.tile_pool(name="wkvp", bufs=1))
        stream = ctx.enter_context(tc.tile_pool(name="stream", bufs=4))
        vtt = ctx.enter_context(tc.tile_pool(name="vtt", bufs=2))
        probs = ctx.enter_context(tc.tile_pool(name="probs", bufs=3))
        attnp = ctx.enter_context(tc.tile_pool(name="attnp", bufs=4))
        outp = ctx.enter_context(tc.tile_pool(name="outp", bufs=4))

        KT = kv.tile([128, S], fmm, tag="KT")        # K^T  [d, k]
        Vn = kv.tile([128, NKT, D], fmm, tag="Vn")   # V natural [k, kt, d]
        QT = qtp.tile([128, NH, SL], fmm)            # Q^T per head [d, h, q]

        ident = misc.tile([128, 128], f32, tag="ident")
        make_identity(nc, ident)
        ones_f = misc.tile([128, 1], f32, tag="ones_f")
        nc.vector.memset(ones_f, 1.0)
        ones_col = misc.tile([128, 1], fmm, tag="ones_col")
        nc.vector.tensor_copy(ones_col, ones_f)
        ones_row = misc.tile([1, 128], f32, tag="ones_row")
        nc.vector.memset(ones_row, 1.0)
        mband = misc.tile([128, 16], f32, tag="mband")
        nc.sync.dma_start(out=mband, in_=mband_d)

        wq_sb = wbig.tile([128, NHT, NH, D], fmm, tag="bigw")
        nc.sync.dma_start(out=wq_sb, in_=wq_r)
        wk_sb = wkvp.tile([128, NHT, D], fmm, tag="wk")
        nc.sync.dma_start(out=wk_sb, in_=wk_r)
        wv_sb = wkvp.tile([128, NHT, D], fmm, tag="wv")
        nc.sync.dma_start(out=wv_sb, in_=wv_r)

        # ---- phase A: Q projection (own rows) ----
        with tc.tile_pool(name="psq", bufs=4, space="PSUM") as psq:
            qps = [psq.tile([128, SL], f32, tag="qps", name=f"qps{h}")
                   for h in range(NH)]
            for i in range(NHT):
                ht = stream.tile([128, SL], fmm, tag="hq")
                nc.sync.dma_start(out=ht, in_=hsQ_r[i])
                for h in range(NH):
                    nc.tensor.matmul(qps[h], wq_sb[:, i, h, :], ht,
                                     start=(i == 0), stop=(i == NHT - 1))
            for h in range(NH):
                nc.vector.tensor_copy(QT[:, h, :], qps[h])

        # ---- phase B: K/V projection (full sequence, streamed) ----
        with tc.tile_pool(name="pskv", bufs=1, space="PSUM") as pskv, \
             tc.tile_pool(name="pst", bufs=2, space="PSUM") as pst:
            for m in range(NSC):
                kps = pskv.tile([128, 512], f32, tag="kps")
                vps = pskv.tile([128, 512], f32, tag="vps")
                for i in range(NHT):
                    ht = stream.tile([128, 512], fmm, tag="hs")
                    nc.sync.dma_start(out=ht, in_=hsT_r[i, :, 512 * m:512 * (m + 1)])
                    nc.tensor.matmul(kps, wk_sb[:, i, :], ht,
                                     start=(i == 0), stop=(i == NHT - 1))
                    nc.tensor.matmul(vps, wv_sb[:, i, :], ht,
                                     start=(i == 0), stop=(i == NHT - 1))
                nc.vector.tensor_copy(KT[:, 512 * m:512 * (m + 1)], kps)
                vt = vtt.tile([128, 512], fmm, tag="vt")
                nc.vector.tensor_copy(vt, vps)
                # full V^T chunk for V_sel host slicing
                nc.sync.dma_start(out=vtf_d[:, 512 * m:512 * (m + 1)], in_=vt)
                # transpose V chunk into natural [k, d] tiles
                for j in range(4):
                    kt_idx = 4 * m + j
                    tps = pst.tile([128, 128], f32, tag="tps")
                    nc.tensor.transpose(
                        tps, vt[:, 128 * j:128 * (j + 1)].bitcast(f32), ident)
                    nc.vector.tensor_copy(Vn[:, kt_idx, :], tps)

        nc.sync.dma_start(out=ktf_d, in_=KT)

        # ---- phase C: attention per head ----
        with tc.tile_pool(name="psst", bufs=2, space="PSUM") as psst, \
             tc.tile_pool(name="pspv", bufs=1, space="PSUM") as pspv, \
             tc.tile_pool(name="psden", bufs=1, space="PSUM") as psden, \
             tc.tile_pool(name="psrb", bufs=1, space="PSUM") as psrb, \
             tc.tile_pool(name="pso", bufs=2, space="PSUM") as pso:
            attnT = []
            for h in range(NH):
                pv = pspv.tile([128, SL], f32, tag="pv")
                den = psden.tile([1, SL], f32, tag="den")
                for kt in range(NKT):
                    if kt < 16:
                        q0, qe = 0, SL          # q-extent [0, 512)
                    else:
                        q0, qe = 256, SL        # q-extent [256, 512)
                    qn = qe - q0
                    jb0 = 16 * kt - q0          # band start, tile-local
                    st = psst.tile([128, SL], f32, tag="st")
                    nc.tensor.matmul(st[:, :qn], KT[:, 128 * kt:128 * (kt + 1)],
                                     QT[:, h, q0:qe], start=True, stop=True)
                    # causal band mask (additive -1e9 on invalid wedge)
                    nc.vector.tensor_tensor(
                        out=st[:, jb0:jb0 + 16], in0=st[:, jb0:jb0 + 16],
                        in1=mband, op=mybir.AluOpType.add)
                    if jb0 > 0:
                        nc.vector.tensor_scalar_add(st[:, :jb0], st[:, :jb0], NEG)
                    pt = probs.tile([128, SL], fmm, tag="pt")
                    nc.scalar.activation(pt[:, :qn], st[:, :qn],
                                         mybir.ActivationFunctionType.Exp,
                                         scale=float(SCALE))
                    nc.tensor.matmul(pv[:, q0:qe], Vn[:, kt, :], pt[:, :qn],
                                     start=(kt == 0), stop=(kt == NKT - 1),
                                     skip_group_check=True)
                    nc.tensor.matmul(den[:, q0:qe], ones_col, pt[:, :qn],
                                     start=(kt == 0), stop=(kt == NKT - 1),
                                     skip_group_check=True)
                # normalize: attnT_norm[d, q] = pv[d, q] / den[q]
                den_sb = misc.tile([1, SL], f32, tag="den_sb")
                nc.vector.tensor_copy(den_sb, den)
                recip = misc.tile([1, SL], f32, tag="recip")
                nc.vector.reciprocal(recip, den_sb)
                rb = psrb.tile([128, SL], f32, tag="rb")
                nc.tensor.matmul(rb, ones_row, recip, start=True, stop=True)
                rb_sb = misc.tile([128, SL], f32, tag="rb_sb")
                nc.vector.tensor_copy(rb_sb, rb)
                at = attnp.tile([128, SL], fmm, tag="at", name=f"at{h}")
                nc.vector.tensor_tensor(out=at, in0=pv, in1=rb_sb,
                                        op=mybir.AluOpType.mult)
                attnT.append(at)

            # ---- phase D: output projection ----
            wo_sb = wbig.tile([128, NH, 8, 512], fmm, tag="bigw")
            nc.sync.dma_start(out=wo_sb, in_=wo_r)
            for t in range(4):
                for n in range(8):
                    ops_ = pso.tile([128, 512], f32, tag="ops")
                    for h in range(NH):
                        nc.tensor.matmul(ops_, attnT[h][:, 128 * t:128 * (t + 1)],
                                         wo_sb[:, h, n, :],
                                         start=(h == 0), stop=(h == NH - 1))
                    osb = outp.tile([128, 512], f32, tag="osb")
                    nc.vector.tensor_copy(osb, ops_)
                    nc.sync.dma_start(
                        out=out_d[128 * t:128 * (t + 1), 512 * n:512 * (n + 1)],
                        in_=osb)

    nc.compile()
    return nc


def _host_prepare(hidden_states, Wq, Wk, Wv, Wo):
    hs = np.ascontiguousarray(
        np.asarray(hidden_states, dtype=np.float32).reshape(S, HID))
    hsT = np.ascontiguousarray(hs.T)                      # [HID, S]
    Wq = np.ascontiguousarray(np.asarray(Wq, dtype=np.float32))
    Wk = np.ascontiguousarray(np.asarray(Wk, dtype=np.float32))
    Wv = np.ascontiguousarray(np.asarray(Wv, dtype=np.float32))
    Wo = np.ascontiguousarray(np.asarray(Wo, dtype=np.float32))
    kp = np.arange(128)[:, None]
    jb = np.arange(16)[None, :]
    in_maps = []
    for c in range(N_CORES):
        hsQ = np.ascontiguousarray(hsT[:, c::N_CORES])    # [HID, SL]
        mband = np.where(kp <= c + 8 * jb, 0.0, NEG).astype(np.float32)
        in_maps.append({
            "hsT": hsT, "hsQ": hsQ, "wq": Wq, "wk": Wk, "wv": Wv, "wo": Wo,
            "mband": mband,
        })
    return in_maps


def kernel(hidden_states, Wq, Wk, Wv, Wo, num_cores_per_group):
    global LAST_PROFILE
    from concourse import bass_utils

    if "nc" not in _CACHE:
        _CACHE["nc"] = _build()
    nc = _CACHE["nc"]

    in_maps = _host_prepare(hidden_states, Wq, Wk, Wv, Wo)
    import os
    trace = bool(int(os.environ.get("FD_KERNEL_TRACE", "0")))
    res = bass_utils.run_bass_kernel_spmd(nc, in_maps, list(range(N_CORES)),
                                          trace=trace)
    if trace:
        LAST_PROFILE = {
            "exec_time_ns": res.exec_time_ns,
            "mean_exec_time_ns": res.mean_exec_time_ns,
            "trace": (res.instructions_and_trace[1]
                      if res.instructions_and_trace else None),
        }

    attn = np.empty((1, S, HID), dtype=np.float32)
    for c in range(N_CORES):
        attn[0, c::N_CORES, :] = res.results[c]["out"]
    ncpg = int(np.asarray(num_cores_per_group))
    K = np.ascontiguousarray(res.results[0]["ktf"].T)   # [S, D]
    V = np.ascontiguousarray(res.results[0]["vtf"].T)
    K_sel = np.ascontiguousarray(K[0::ncpg])[None, None]
    V_sel = np.ascontiguousarray(V[0::ncpg])[None, None]
    return attn, K_sel, V_sel
